# revision 10
# baseline (speedup 1.0000x reference)
"""Block-circulant linear layer as a dense matmul on TRN2.

y[n, j*B+k] = sum_{i,b} c[j,i,(k-b) mod B] * x[n, i*B+b] + bias[j*B+k]

Equivalent to y = x @ W + bias with W[i*B+b, j*B+k] = c[j,i,(k-b)%B],
a 4096x4096 block-circulant matrix materialized on host.

Sharding: data-parallel over the 8192 tokens (1024 tokens/core); W and
bias replicated. Per core: (1024,4096) @ (4096,4096) fp32r matmul,
x shipped pre-transposed (infeat-major) so token tiles are direct
stationary operands.
"""

import numpy as np

import concourse.bass as bass
import concourse.mybir as mybir
import concourse.tile as tile
from concourse import bacc
from concourse.bass_utils import run_bass_kernel_spmd

B = 256
IN_BLOCKS = 16
OUT_BLOCKS = 16
BATCH, SEQ = 4, 2048
IN_F = IN_BLOCKS * B     # 4096
OUT_F = OUT_BLOCKS * B   # 4096
N_CORES = 8
NTOK = BATCH * SEQ       # 8192
TOK = NTOK // N_CORES    # 1024 tokens per core

KT = IN_F // 128         # 32 contraction tiles
MT = TOK // 128          # 8 token tiles
NW = 512                 # moving free dim per matmul (fp32 max, 1 psum bank)
NT = OUT_F // NW         # 8 outfeat chunks

_NC_CACHE = {}


def _build_nc():
    f32 = mybir.dt.float32
    f32r = mybir.dt.float32r

    nc = bacc.Bacc("TRN2", target_bir_lowering=False, debug=False)
    xT = nc.dram_tensor("xT", [IN_F, TOK], f32r, kind="ExternalInput")
    w = nc.dram_tensor("w", [IN_F, OUT_F], f32r, kind="ExternalInput")
    biasb = nc.dram_tensor("biasb", [128, OUT_F], f32, kind="ExternalInput")
    y = nc.dram_tensor("y", [TOK, OUT_F], f32, kind="ExternalOutput")

    with tile.TileContext(nc) as tc:
        with (
            tc.tile_pool(name="xpool", bufs=1) as xpool,
            tc.tile_pool(name="bpool", bufs=1) as bpool,
            tc.tile_pool(name="wpool", bufs=8) as wpool,
            tc.tile_pool(name="ypool", bufs=4) as ypool,
            tc.tile_pool(name="psum", bufs=8, space="PSUM") as psum_pool,
        ):
            # Resident: all 32 k-tiles of x^T (16MB) + broadcast bias (2MB).
            xts = []
            for k in range(KT):
                xt = xpool.tile([128, TOK], f32r, tag=f"x{k}")
                nc.sync.dma_start(out=xt[:], in_=xT[k * 128 : (k + 1) * 128, :])
                xts.append(xt)
            bias_sb = bpool.tile([128, OUT_F], f32, tag="bias")
            nc.sync.dma_start(out=bias_sb[:], in_=biasb[:, :])

            for n in range(NT):
                nsl = slice(n * NW, (n + 1) * NW)
                psums = [
                    psum_pool.tile([128, NW], f32, tag="ps", name=f"ps_{n}_{m}")
                    for m in range(MT)
                ]
                for k in range(KT):
                    wt = wpool.tile([128, NW], f32r, tag="w")
                    nc.sync.dma_start(
                        out=wt[:], in_=w[k * 128 : (k + 1) * 128, nsl]
                    )
                    for m in range(MT):
                        nc.tensor.matmul(
                            psums[m][:],
                            xts[k][:, m * 128 : (m + 1) * 128],
                            wt[:],
                            start=(k == 0),
                            stop=(k == KT - 1),
                        )
                for m in range(MT):
                    yt = ypool.tile([128, NW], f32, tag="y")
                    nc.vector.tensor_add(yt[:], psums[m][:], bias_sb[:, nsl])
                    nc.sync.dma_start(
                        out=y[m * 128 : (m + 1) * 128, nsl], in_=yt[:]
                    )
    nc.finalize()
    return nc


def _get_nc():
    if "nc" not in _NC_CACHE:
        _NC_CACHE["nc"] = _build_nc()
    return _NC_CACHE["nc"]


def _round_fp32r(a: np.ndarray) -> np.ndarray:
    """Round fp32 to fp32r (e8m11: low 12 mantissa bits zero), RNE."""
    u = np.ascontiguousarray(a, dtype=np.float32).view(np.uint32)
    r = (u + (0x7FF + ((u >> 12) & 1))) & np.uint32(0xFFFFF000)
    return r.view(np.float32)


def _build_weight(c: np.ndarray) -> np.ndarray:
    # W[i*B+b, j*B+k] = c[j, i, (k-b) mod B]
    k = np.arange(B)
    b = np.arange(B)
    idx = (k[None, :] - b[:, None]) % B                    # (b, k)
    blocks = c[:, :, idx]                                  # (j, i, b, k)
    return np.ascontiguousarray(
        blocks.transpose(1, 2, 0, 3).reshape(IN_F, OUT_F), dtype=np.float32
    )


def kernel(x, c, bias, _spmd_kwargs=None):
    x = np.asarray(x, dtype=np.float32)
    c = np.asarray(c, dtype=np.float32)
    bias = np.asarray(bias, dtype=np.float32)

    w = _round_fp32r(_build_weight(c))
    biasb = np.ascontiguousarray(np.broadcast_to(bias, (128, OUT_F)))
    xf = x.reshape(NTOK, IN_F)

    in_maps = []
    for cid in range(N_CORES):
        shard = xf[cid * TOK : (cid + 1) * TOK]            # (TOK, IN_F)
        in_maps.append(
            {
                "xT": _round_fp32r(shard.T),               # (IN_F, TOK)
                "w": w,
                "biasb": biasb,
            }
        )

    nc = _get_nc()
    kw = dict(_spmd_kwargs or {})
    one_core = kw.pop("_one_core", False)
    if one_core:
        res = run_bass_kernel_spmd(nc, in_maps[:1], core_ids=[0], **kw)
        return None, res
    res = run_bass_kernel_spmd(
        nc, in_maps, core_ids=list(range(N_CORES)), **kw
    )
    y = np.concatenate([r["y"] for r in res.results], axis=0)  # (NTOK, OUT_F)
    out = y.reshape(BATCH, SEQ, OUT_F)
    if _spmd_kwargs:
        return out, res
    return out


# revision 13
# speedup vs baseline: 1.5117x; 1.5117x over previous
"""Block-circulant linear layer on TRN2 via one-level circulant CRT split.

y[n, j*B+k] = sum_{i,b} c[j,i,(k-b) mod B] * x[n, i*B+b] + bias[j*B+k]

Using x^256-1 = (x^128-1)(x^128+1): with u_i = x_i[:128]+x_i[128:],
v_i = x_i[:128]-x_i[128:], the op becomes two half-size dense systems
  yu = u @ U/2 + beta_u/2   (U: cyclic-128 block matrix, 2048x2048)
  yv = v @ V/2 + beta_v/2   (V: negacyclic-128 block matrix)
  y_lo = yu + yv, y_hi = yu - yv
— half the matmul FLOPs of the dense 4096x4096 form.

Sharding: data-parallel over the 8192 tokens (1024/core); U,V replicated.
fp32r (e8m11) matmul datapath at full PE rate; bias folded in via a K=1
ones-row matmul.
"""

import numpy as np

import concourse.bass as bass
import concourse.mybir as mybir
import concourse.tile as tile
from concourse import bacc
from concourse.bass_utils import run_bass_kernel_spmd

B = 256
H = B // 2               # 128
IN_BLOCKS = 16
OUT_BLOCKS = 16
BATCH, SEQ = 4, 2048
IN_F = IN_BLOCKS * B     # 4096
OUT_F = OUT_BLOCKS * B   # 4096
HF = IN_BLOCKS * H       # 2048 (half-system width)
N_CORES = 8
NTOK = BATCH * SEQ       # 8192
TOK = NTOK // N_CORES    # 1024 tokens per core

KT = HF // 128           # 16 contraction tiles per system
MT = TOK // 128          # 8 token tiles
NW = 512                 # moving free dim per matmul (one psum bank)
NT = HF // NW            # 4 column chunks per system
JB = NW // H             # 4 j-blocks per column chunk

_NC_CACHE = {}


def _build_nc():
    f32 = mybir.dt.float32
    f32r = mybir.dt.float32r

    nc = bacc.Bacc("TRN2", target_bir_lowering=False, debug=False)
    uT = nc.dram_tensor("uT", [HF, TOK], f32r, kind="ExternalInput")
    vT = nc.dram_tensor("vT", [HF, TOK], f32r, kind="ExternalInput")
    wU = nc.dram_tensor("wU", [HF, HF], f32r, kind="ExternalInput")
    wV = nc.dram_tensor("wV", [HF, HF], f32r, kind="ExternalInput")
    betaU = nc.dram_tensor("betaU", [1, HF], f32r, kind="ExternalInput")
    betaV = nc.dram_tensor("betaV", [1, HF], f32r, kind="ExternalInput")
    ones = nc.dram_tensor("ones", [1, TOK], f32r, kind="ExternalInput")
    # y as (tok, j, lo/hi, kk) so the strided recombine store is a plain AP
    y = nc.dram_tensor(
        "y", [TOK, OUT_BLOCKS, 2, H], f32, kind="ExternalOutput"
    )

    with tile.TileContext(nc) as tc:
        with (
            tc.tile_pool(name="uvpool", bufs=1) as uvpool,
            tc.tile_pool(name="cpool", bufs=1) as cpool,
            tc.tile_pool(name="wpool", bufs=6) as wpool,
            tc.tile_pool(name="epool", bufs=8) as epool,
            tc.tile_pool(name="ypool", bufs=2) as ypool,
            tc.tile_pool(name="psum", bufs=8, space="PSUM") as psum_pool,
        ):
            # Resident u/v k-tiles (host-computed butterfly, feat-major)
            us, vs = [], []
            for i in range(IN_BLOCKS):
                ut = uvpool.tile([128, TOK], f32r, tag=f"u{i}", name=f"u{i}")
                vt = uvpool.tile([128, TOK], f32r, tag=f"v{i}", name=f"v{i}")
                nc.sync.dma_start(out=ut[:], in_=uT[i * 128 : (i + 1) * 128, :])
                nc.sync.dma_start(out=vt[:], in_=vT[i * 128 : (i + 1) * 128, :])
                us.append(ut)
                vs.append(vt)

            ones_sb = cpool.tile([1, TOK], f32r, tag="ones")
            nc.sync.dma_start(out=ones_sb[:], in_=ones[:, :])
            betaU_sb = cpool.tile([1, HF], f32r, tag="bU")
            nc.sync.dma_start(out=betaU_sb[:], in_=betaU[:, :])
            betaV_sb = cpool.tile([1, HF], f32r, tag="bV")
            nc.sync.dma_start(out=betaV_sb[:], in_=betaV[:, :])

            for n in range(NT):
                nsl = slice(n * NW, (n + 1) * NW)
                # --- U phase ---
                psU = [
                    psum_pool.tile([128, NW], f32, tag="ps", name=f"pu_{n}_{m}")
                    for m in range(MT)
                ]
                for k in range(KT):
                    wt = wpool.tile([128, NW], f32r, tag="w", name=f"wu_{n}_{k}")
                    nc.sync.dma_start(out=wt[:], in_=wU[k * 128 : (k + 1) * 128, nsl])
                    for m in range(MT):
                        nc.tensor.matmul(
                            psU[m][:],
                            us[k][:, m * 128 : (m + 1) * 128],
                            wt[:],
                            start=(k == 0),
                            stop=False,
                        )
                for m in range(MT):
                    nc.tensor.matmul(
                        psU[m][:],
                        ones_sb[:, m * 128 : (m + 1) * 128],
                        betaU_sb[:, nsl],
                        start=False,
                        stop=True,
                    )
                yus = []
                for m in range(MT):
                    yu = epool.tile([128, NW], f32, tag="yu", name=f"yu_{n}_{m}")
                    nc.vector.tensor_copy(yu[:], psU[m][:])
                    yus.append(yu)
                # --- V phase ---
                psV = [
                    psum_pool.tile([128, NW], f32, tag="ps", name=f"pv_{n}_{m}")
                    for m in range(MT)
                ]
                for k in range(KT):
                    wt = wpool.tile([128, NW], f32r, tag="w", name=f"wv_{n}_{k}")
                    nc.sync.dma_start(out=wt[:], in_=wV[k * 128 : (k + 1) * 128, nsl])
                    for m in range(MT):
                        nc.tensor.matmul(
                            psV[m][:],
                            vs[k][:, m * 128 : (m + 1) * 128],
                            wt[:],
                            start=(k == 0),
                            stop=False,
                        )
                for m in range(MT):
                    nc.tensor.matmul(
                        psV[m][:],
                        ones_sb[:, m * 128 : (m + 1) * 128],
                        betaV_sb[:, nsl],
                        start=False,
                        stop=True,
                    )
                # --- recombine + store ---
                for m in range(MT):
                    tlo = ypool.tile([128, NW], f32, tag="tlo", name=f"tlo_{n}_{m}")
                    thi = ypool.tile([128, NW], f32, tag="thi", name=f"thi_{n}_{m}")
                    nc.vector.tensor_add(tlo[:], yus[m][:], psV[m][:])
                    nc.vector.tensor_sub(thi[:], yus[m][:], psV[m][:])
                    rows = slice(m * 128, (m + 1) * 128)
                    jsl = slice(n * JB, (n + 1) * JB)
                    nc.sync.dma_start(out=y[rows, jsl, 0, :], in_=tlo[:])
                    nc.sync.dma_start(out=y[rows, jsl, 1, :], in_=thi[:])
    nc.finalize()
    return nc


def _get_nc():
    if "nc" not in _NC_CACHE:
        _NC_CACHE["nc"] = _build_nc()
    return _NC_CACHE["nc"]


def _round_fp32r(a: np.ndarray) -> np.ndarray:
    """Round fp32 to fp32r (e8m11: low 12 mantissa bits zero), RNE."""
    u = np.ascontiguousarray(a, dtype=np.float32).view(np.uint32)
    r = (u + (0x7FF + ((u >> 12) & 1))) & np.uint32(0xFFFFF000)
    return r.view(np.float32)


def _build_weights(c: np.ndarray, bias: np.ndarray):
    # cyclic/negacyclic half-size blocks
    cu = c[:, :, :H] + c[:, :, H:]                         # (J, I, H)
    cv = c[:, :, :H] - c[:, :, H:]
    kk = np.arange(H)
    bb = np.arange(H)
    idx = (kk[None, :] - bb[:, None]) % H                  # (bb, kk)
    sign = np.where(kk[None, :] >= bb[:, None], 1.0, -1.0).astype(np.float32)
    U = cu[:, :, idx].transpose(1, 2, 0, 3).reshape(HF, HF) * 0.5
    V = (cv[:, :, idx] * sign[None, None]).transpose(1, 2, 0, 3).reshape(
        HF, HF
    ) * 0.5
    bias_b = bias.reshape(OUT_BLOCKS, B)
    beta_u = 0.5 * (bias_b[:, :H] + bias_b[:, H:]).reshape(1, HF)
    beta_v = 0.5 * (bias_b[:, :H] - bias_b[:, H:]).reshape(1, HF)
    return (
        _round_fp32r(U),
        _round_fp32r(V),
        _round_fp32r(beta_u),
        _round_fp32r(beta_v),
    )


def kernel(x, c, bias, _spmd_kwargs=None):
    x = np.asarray(x, dtype=np.float32)
    c = np.asarray(c, dtype=np.float32)
    bias = np.asarray(bias, dtype=np.float32)

    wu, wv, bu, bv = _build_weights(c, bias)
    ones = np.ones((1, TOK), dtype=np.float32)
    xb = x.reshape(NTOK, IN_BLOCKS, B)
    u_all = (xb[:, :, :H] + xb[:, :, H:]).reshape(NTOK, HF)
    v_all = (xb[:, :, :H] - xb[:, :, H:]).reshape(NTOK, HF)

    in_maps = []
    for cid in range(N_CORES):
        sl = slice(cid * TOK, (cid + 1) * TOK)
        in_maps.append(
            {
                "uT": _round_fp32r(u_all[sl].T),           # (HF, TOK)
                "vT": _round_fp32r(v_all[sl].T),
                "wU": wu,
                "wV": wv,
                "betaU": bu,
                "betaV": bv,
                "ones": ones,
            }
        )

    nc = _get_nc()
    kw = dict(_spmd_kwargs or {})
    one_core = kw.pop("_one_core", False)
    if one_core:
        res = run_bass_kernel_spmd(nc, in_maps[:1], core_ids=[0], **kw)
        return None, res
    res = run_bass_kernel_spmd(
        nc, in_maps, core_ids=list(range(N_CORES)), **kw
    )
    y = np.concatenate(
        [r["y"].reshape(TOK, OUT_F) for r in res.results], axis=0
    )
    out = y.reshape(BATCH, SEQ, OUT_F)
    if _spmd_kwargs:
        return out, res
    return out


# revision 14
# speedup vs baseline: 1.5804x; 1.0455x over previous
"""Block-circulant linear layer on TRN2 via one-level circulant CRT split.

y[n, j*B+k] = sum_{i,b} c[j,i,(k-b) mod B] * x[n, i*B+b] + bias[j*B+k]

Using x^256-1 = (x^128-1)(x^128+1): with u_i = x_i[:128]+x_i[128:],
v_i = x_i[:128]-x_i[128:], the op becomes two half-size dense systems
  yu = u @ U/2 + beta_u/2   (U: cyclic-128 block matrix, 2048x2048)
  yv = v @ V/2 + beta_v/2   (V: negacyclic-128 block matrix)
  y_lo = yu + yv, y_hi = yu - yv
— half the matmul FLOPs of the dense 4096x4096 form.

Sharding: data-parallel over the 8192 tokens (1024/core); U,V replicated.
fp32r (e8m11) matmul datapath at full PE rate; bias folded in via a K=1
ones-row matmul.
"""

import numpy as np

import concourse.bass as bass
import concourse.mybir as mybir
import concourse.tile as tile
from concourse import bacc
from concourse.bass_utils import run_bass_kernel_spmd

B = 256
H = B // 2               # 128
IN_BLOCKS = 16
OUT_BLOCKS = 16
BATCH, SEQ = 4, 2048
IN_F = IN_BLOCKS * B     # 4096
OUT_F = OUT_BLOCKS * B   # 4096
HF = IN_BLOCKS * H       # 2048 (half-system width)
N_CORES = 8
NTOK = BATCH * SEQ       # 8192
TOK = NTOK // N_CORES    # 1024 tokens per core

KT = HF // 128           # 16 contraction tiles per system
MT = TOK // 128          # 8 token tiles
NW = 512                 # moving free dim per matmul (one psum bank)
NT = HF // NW            # 4 column chunks per system
JB = NW // H             # 4 j-blocks per column chunk

_NC_CACHE = {}


def _build_nc():
    f32 = mybir.dt.float32
    f32r = mybir.dt.float32r

    nc = bacc.Bacc("TRN2", target_bir_lowering=False, debug=False)
    uT = nc.dram_tensor("uT", [HF, TOK], f32r, kind="ExternalInput")
    vT = nc.dram_tensor("vT", [HF, TOK], f32r, kind="ExternalInput")
    wU = nc.dram_tensor("wU", [NT, KT, 128, NW], f32r, kind="ExternalInput")
    wV = nc.dram_tensor("wV", [NT, KT, 128, NW], f32r, kind="ExternalInput")
    betaU = nc.dram_tensor("betaU", [1, HF], f32r, kind="ExternalInput")
    betaV = nc.dram_tensor("betaV", [1, HF], f32r, kind="ExternalInput")
    ones = nc.dram_tensor("ones", [1, TOK], f32r, kind="ExternalInput")
    # y stored as raw tiles (n, m, lo/hi, 128, NW); host reassembles
    y = nc.dram_tensor(
        "y", [NT, MT, 2, 128, NW], f32, kind="ExternalOutput"
    )

    with tile.TileContext(nc) as tc:
        with (
            tc.tile_pool(name="uvpool", bufs=1) as uvpool,
            tc.tile_pool(name="cpool", bufs=1) as cpool,
            tc.tile_pool(name="wpool", bufs=6) as wpool,
            tc.tile_pool(name="epool", bufs=8) as epool,
            tc.tile_pool(name="ypool", bufs=2) as ypool,
            tc.tile_pool(name="psum", bufs=8, space="PSUM") as psum_pool,
        ):
            # Resident u/v k-tiles (host-computed butterfly, feat-major)
            us, vs = [], []
            for i in range(IN_BLOCKS):
                ut = uvpool.tile([128, TOK], f32r, tag=f"u{i}", name=f"u{i}")
                vt = uvpool.tile([128, TOK], f32r, tag=f"v{i}", name=f"v{i}")
                nc.sync.dma_start(out=ut[:], in_=uT[i * 128 : (i + 1) * 128, :])
                nc.sync.dma_start(out=vt[:], in_=vT[i * 128 : (i + 1) * 128, :])
                us.append(ut)
                vs.append(vt)

            ones_sb = cpool.tile([1, TOK], f32r, tag="ones")
            nc.sync.dma_start(out=ones_sb[:], in_=ones[:, :])
            betaU_sb = cpool.tile([1, HF], f32r, tag="bU")
            nc.sync.dma_start(out=betaU_sb[:], in_=betaU[:, :])
            betaV_sb = cpool.tile([1, HF], f32r, tag="bV")
            nc.sync.dma_start(out=betaV_sb[:], in_=betaV[:, :])

            for n in range(NT):
                nsl = slice(n * NW, (n + 1) * NW)
                # --- U phase ---
                psU = [
                    psum_pool.tile([128, NW], f32, tag="ps", name=f"pu_{n}_{m}")
                    for m in range(MT)
                ]
                for k in range(KT):
                    wt = wpool.tile([128, NW], f32r, tag="w", name=f"wu_{n}_{k}")
                    nc.sync.dma_start(out=wt[:], in_=wU[n, k, :, :])
                    for m in range(MT):
                        nc.tensor.matmul(
                            psU[m][:],
                            us[k][:, m * 128 : (m + 1) * 128],
                            wt[:],
                            start=(k == 0),
                            stop=False,
                        )
                for m in range(MT):
                    nc.tensor.matmul(
                        psU[m][:],
                        ones_sb[:, m * 128 : (m + 1) * 128],
                        betaU_sb[:, nsl],
                        start=False,
                        stop=True,
                    )
                yus = []
                for m in range(MT):
                    yu = epool.tile([128, NW], f32, tag="yu", name=f"yu_{n}_{m}")
                    nc.vector.tensor_copy(yu[:], psU[m][:])
                    yus.append(yu)
                # --- V phase ---
                psV = [
                    psum_pool.tile([128, NW], f32, tag="ps", name=f"pv_{n}_{m}")
                    for m in range(MT)
                ]
                for k in range(KT):
                    wt = wpool.tile([128, NW], f32r, tag="w", name=f"wv_{n}_{k}")
                    nc.sync.dma_start(out=wt[:], in_=wV[n, k, :, :])
                    for m in range(MT):
                        nc.tensor.matmul(
                            psV[m][:],
                            vs[k][:, m * 128 : (m + 1) * 128],
                            wt[:],
                            start=(k == 0),
                            stop=False,
                        )
                for m in range(MT):
                    nc.tensor.matmul(
                        psV[m][:],
                        ones_sb[:, m * 128 : (m + 1) * 128],
                        betaV_sb[:, nsl],
                        start=False,
                        stop=True,
                    )
                # --- recombine + store ---
                for m in range(MT):
                    tlo = ypool.tile([128, NW], f32, tag="tlo", name=f"tlo_{n}_{m}")
                    thi = ypool.tile([128, NW], f32, tag="thi", name=f"thi_{n}_{m}")
                    nc.vector.tensor_add(tlo[:], yus[m][:], psV[m][:])
                    nc.vector.tensor_sub(thi[:], yus[m][:], psV[m][:])
                    nc.sync.dma_start(out=y[n, m, 0, :, :], in_=tlo[:])
                    nc.sync.dma_start(out=y[n, m, 1, :, :], in_=thi[:])
    nc.finalize()
    return nc


def _get_nc():
    if "nc" not in _NC_CACHE:
        _NC_CACHE["nc"] = _build_nc()
    return _NC_CACHE["nc"]


def _round_fp32r(a: np.ndarray) -> np.ndarray:
    """Round fp32 to fp32r (e8m11: low 12 mantissa bits zero), RNE."""
    u = np.ascontiguousarray(a, dtype=np.float32).view(np.uint32)
    r = (u + (0x7FF + ((u >> 12) & 1))) & np.uint32(0xFFFFF000)
    return r.view(np.float32)


def _build_weights(c: np.ndarray, bias: np.ndarray):
    # cyclic/negacyclic half-size blocks
    cu = c[:, :, :H] + c[:, :, H:]                         # (J, I, H)
    cv = c[:, :, :H] - c[:, :, H:]
    kk = np.arange(H)
    bb = np.arange(H)
    idx = (kk[None, :] - bb[:, None]) % H                  # (bb, kk)
    sign = np.where(kk[None, :] >= bb[:, None], 1.0, -1.0).astype(np.float32)
    U = cu[:, :, idx].transpose(1, 2, 0, 3).reshape(HF, HF) * 0.5
    V = (cv[:, :, idx] * sign[None, None]).transpose(1, 2, 0, 3).reshape(
        HF, HF
    ) * 0.5
    bias_b = bias.reshape(OUT_BLOCKS, B)
    beta_u = 0.5 * (bias_b[:, :H] + bias_b[:, H:]).reshape(1, HF)
    beta_v = 0.5 * (bias_b[:, :H] - bias_b[:, H:]).reshape(1, HF)
    def tiled(w):
        # (HF, HF) -> (NT, KT, 128, NW) so each [128, NW] tile is contiguous
        return np.ascontiguousarray(
            w.reshape(KT, 128, NT, NW).transpose(2, 0, 1, 3)
        )

    return (
        _round_fp32r(tiled(U)),
        _round_fp32r(tiled(V)),
        _round_fp32r(beta_u),
        _round_fp32r(beta_v),
    )


def kernel(x, c, bias, _spmd_kwargs=None):
    x = np.asarray(x, dtype=np.float32)
    c = np.asarray(c, dtype=np.float32)
    bias = np.asarray(bias, dtype=np.float32)

    wu, wv, bu, bv = _build_weights(c, bias)
    ones = np.ones((1, TOK), dtype=np.float32)
    xb = x.reshape(NTOK, IN_BLOCKS, B)
    u_all = (xb[:, :, :H] + xb[:, :, H:]).reshape(NTOK, HF)
    v_all = (xb[:, :, :H] - xb[:, :, H:]).reshape(NTOK, HF)

    in_maps = []
    for cid in range(N_CORES):
        sl = slice(cid * TOK, (cid + 1) * TOK)
        in_maps.append(
            {
                "uT": _round_fp32r(u_all[sl].T),           # (HF, TOK)
                "vT": _round_fp32r(v_all[sl].T),
                "wU": wu,
                "wV": wv,
                "betaU": bu,
                "betaV": bv,
                "ones": ones,
            }
        )

    nc = _get_nc()
    kw = dict(_spmd_kwargs or {})
    one_core = kw.pop("_one_core", False)
    if one_core:
        res = run_bass_kernel_spmd(nc, in_maps[:1], core_ids=[0], **kw)
        return None, res
    res = run_bass_kernel_spmd(
        nc, in_maps, core_ids=list(range(N_CORES)), **kw
    )
    def reassemble(a):
        # (NT, MT, 2, 128, NW) -> (TOK, OUT_F)
        a = a.reshape(NT, MT, 2, 128, JB, H)
        return a.transpose(1, 3, 0, 4, 2, 5).reshape(TOK, OUT_F)

    y = np.concatenate([reassemble(r["y"]) for r in res.results], axis=0)
    out = y.reshape(BATCH, SEQ, OUT_F)
    if _spmd_kwargs:
        return out, res
    return out


# revision 15
# speedup vs baseline: 1.6164x; 1.0227x over previous
"""Block-circulant linear layer on TRN2 via one-level circulant CRT split.

y[n, j*B+k] = sum_{i,b} c[j,i,(k-b) mod B] * x[n, i*B+b] + bias[j*B+k]

Using x^256-1 = (x^128-1)(x^128+1): with u_i = x_i[:128]+x_i[128:],
v_i = x_i[:128]-x_i[128:], the op becomes two half-size dense systems
  yu = u @ U/2 + beta_u/2   (U: cyclic-128 block matrix, 2048x2048)
  yv = v @ V/2 + beta_v/2   (V: negacyclic-128 block matrix)
  y_lo = yu + yv, y_hi = yu - yv
— half the matmul FLOPs of the dense 4096x4096 form.

Sharding: data-parallel over the 8192 tokens (1024/core); U,V replicated.
fp32r (e8m11) matmul datapath at full PE rate; bias folded in via a K=1
ones-row matmul.
"""

import numpy as np

import concourse.bass as bass
import concourse.mybir as mybir
import concourse.tile as tile
from concourse import bacc
from concourse.bass_utils import run_bass_kernel_spmd

B = 256
H = B // 2               # 128
IN_BLOCKS = 16
OUT_BLOCKS = 16
BATCH, SEQ = 4, 2048
IN_F = IN_BLOCKS * B     # 4096
OUT_F = OUT_BLOCKS * B   # 4096
HF = IN_BLOCKS * H       # 2048 (half-system width)
N_CORES = 8
NTOK = BATCH * SEQ       # 8192
TOK = NTOK // N_CORES    # 1024 tokens per core

KT = HF // 128           # 16 contraction tiles per system
MT = TOK // 128          # 8 token tiles
NW = 512                 # moving free dim per matmul (one psum bank)
NT = HF // NW            # 4 column chunks per system
JB = NW // H             # 4 j-blocks per column chunk

_NC_CACHE = {}


def _build_nc():
    f32 = mybir.dt.float32
    f32r = mybir.dt.float32r

    nc = bacc.Bacc("TRN2", target_bir_lowering=False, debug=False)
    uT = nc.dram_tensor("uT", [HF, TOK], f32r, kind="ExternalInput")
    vT = nc.dram_tensor("vT", [HF, TOK], f32r, kind="ExternalInput")
    wU = nc.dram_tensor("wU", [NT, KT, 128, NW], f32r, kind="ExternalInput")
    wV = nc.dram_tensor("wV", [NT, KT, 128, NW], f32r, kind="ExternalInput")
    betaU = nc.dram_tensor("betaU", [1, HF], f32r, kind="ExternalInput")
    betaV = nc.dram_tensor("betaV", [1, HF], f32r, kind="ExternalInput")
    ones = nc.dram_tensor("ones", [1, TOK], f32r, kind="ExternalInput")
    # y stored as raw tiles (n, m, lo/hi, 128, NW); host reassembles
    y = nc.dram_tensor(
        "y", [NT, MT, 2, 128, NW], f32, kind="ExternalOutput"
    )

    with tile.TileContext(nc) as tc:
        with (
            tc.tile_pool(name="uvpool", bufs=1) as uvpool,
            tc.tile_pool(name="cpool", bufs=1) as cpool,
            tc.tile_pool(name="wpool", bufs=6) as wpool,
            tc.tile_pool(name="epool", bufs=8) as epool,
            tc.tile_pool(name="ypool", bufs=2) as ypool,
            tc.tile_pool(name="psum", bufs=8, space="PSUM") as psum_pool,
        ):
            # Resident u/v k-tiles (host-computed butterfly, feat-major)
            us, vs = [], []
            for i in range(IN_BLOCKS):
                ut = uvpool.tile([128, TOK], f32r, tag=f"u{i}", name=f"u{i}")
                vt = uvpool.tile([128, TOK], f32r, tag=f"v{i}", name=f"v{i}")
                nc.sync.dma_start(out=ut[:], in_=uT[i * 128 : (i + 1) * 128, :])
                nc.sync.dma_start(out=vt[:], in_=vT[i * 128 : (i + 1) * 128, :])
                us.append(ut)
                vs.append(vt)

            ones_sb = cpool.tile([1, TOK], f32r, tag="ones")
            nc.sync.dma_start(out=ones_sb[:], in_=ones[:, :])
            betaU_sb = cpool.tile([1, HF], f32r, tag="bU")
            nc.sync.dma_start(out=betaU_sb[:], in_=betaU[:, :])
            betaV_sb = cpool.tile([1, HF], f32r, tag="bV")
            nc.sync.dma_start(out=betaV_sb[:], in_=betaV[:, :])

            for n in range(NT):
                nsl = slice(n * NW, (n + 1) * NW)
                # --- U phase ---
                psU = [
                    psum_pool.tile([128, NW], f32, tag="ps", name=f"pu_{n}_{m}")
                    for m in range(MT)
                ]
                for k in range(KT):
                    wt = wpool.tile([128, NW], f32r, tag="w", name=f"wu_{n}_{k}")
                    nc.gpsimd.dma_start(out=wt[:], in_=wU[n, k, :, :])
                    for m in range(MT):
                        nc.tensor.matmul(
                            psU[m][:],
                            us[k][:, m * 128 : (m + 1) * 128],
                            wt[:],
                            start=(k == 0),
                            stop=False,
                        )
                for m in range(MT):
                    nc.tensor.matmul(
                        psU[m][:],
                        ones_sb[:, m * 128 : (m + 1) * 128],
                        betaU_sb[:, nsl],
                        start=False,
                        stop=True,
                    )
                yus = []
                for m in range(MT):
                    yu = epool.tile([128, NW], f32, tag="yu", name=f"yu_{n}_{m}")
                    nc.vector.tensor_copy(yu[:], psU[m][:])
                    yus.append(yu)
                # --- V phase ---
                psV = [
                    psum_pool.tile([128, NW], f32, tag="ps", name=f"pv_{n}_{m}")
                    for m in range(MT)
                ]
                for k in range(KT):
                    wt = wpool.tile([128, NW], f32r, tag="w", name=f"wv_{n}_{k}")
                    nc.gpsimd.dma_start(out=wt[:], in_=wV[n, k, :, :])
                    for m in range(MT):
                        nc.tensor.matmul(
                            psV[m][:],
                            vs[k][:, m * 128 : (m + 1) * 128],
                            wt[:],
                            start=(k == 0),
                            stop=False,
                        )
                for m in range(MT):
                    nc.tensor.matmul(
                        psV[m][:],
                        ones_sb[:, m * 128 : (m + 1) * 128],
                        betaV_sb[:, nsl],
                        start=False,
                        stop=True,
                    )
                # --- recombine + store ---
                for m in range(MT):
                    tlo = ypool.tile([128, NW], f32, tag="tlo", name=f"tlo_{n}_{m}")
                    thi = ypool.tile([128, NW], f32, tag="thi", name=f"thi_{n}_{m}")
                    nc.vector.tensor_add(tlo[:], yus[m][:], psV[m][:])
                    nc.vector.tensor_sub(thi[:], yus[m][:], psV[m][:])
                    nc.sync.dma_start(out=y[n, m, 0, :, :], in_=tlo[:])
                    nc.sync.dma_start(out=y[n, m, 1, :, :], in_=thi[:])
    nc.finalize()
    return nc


def _get_nc():
    if "nc" not in _NC_CACHE:
        _NC_CACHE["nc"] = _build_nc()
    return _NC_CACHE["nc"]


def _round_fp32r(a: np.ndarray) -> np.ndarray:
    """Round fp32 to fp32r (e8m11: low 12 mantissa bits zero), RNE."""
    u = np.ascontiguousarray(a, dtype=np.float32).view(np.uint32)
    r = (u + (0x7FF + ((u >> 12) & 1))) & np.uint32(0xFFFFF000)
    return r.view(np.float32)


def _build_weights(c: np.ndarray, bias: np.ndarray):
    # cyclic/negacyclic half-size blocks
    cu = c[:, :, :H] + c[:, :, H:]                         # (J, I, H)
    cv = c[:, :, :H] - c[:, :, H:]
    kk = np.arange(H)
    bb = np.arange(H)
    idx = (kk[None, :] - bb[:, None]) % H                  # (bb, kk)
    sign = np.where(kk[None, :] >= bb[:, None], 1.0, -1.0).astype(np.float32)
    U = cu[:, :, idx].transpose(1, 2, 0, 3).reshape(HF, HF) * 0.5
    V = (cv[:, :, idx] * sign[None, None]).transpose(1, 2, 0, 3).reshape(
        HF, HF
    ) * 0.5
    bias_b = bias.reshape(OUT_BLOCKS, B)
    beta_u = 0.5 * (bias_b[:, :H] + bias_b[:, H:]).reshape(1, HF)
    beta_v = 0.5 * (bias_b[:, :H] - bias_b[:, H:]).reshape(1, HF)
    def tiled(w):
        # (HF, HF) -> (NT, KT, 128, NW) so each [128, NW] tile is contiguous
        return np.ascontiguousarray(
            w.reshape(KT, 128, NT, NW).transpose(2, 0, 1, 3)
        )

    return (
        _round_fp32r(tiled(U)),
        _round_fp32r(tiled(V)),
        _round_fp32r(beta_u),
        _round_fp32r(beta_v),
    )


def kernel(x, c, bias, _spmd_kwargs=None):
    x = np.asarray(x, dtype=np.float32)
    c = np.asarray(c, dtype=np.float32)
    bias = np.asarray(bias, dtype=np.float32)

    wu, wv, bu, bv = _build_weights(c, bias)
    ones = np.ones((1, TOK), dtype=np.float32)
    xb = x.reshape(NTOK, IN_BLOCKS, B)
    u_all = (xb[:, :, :H] + xb[:, :, H:]).reshape(NTOK, HF)
    v_all = (xb[:, :, :H] - xb[:, :, H:]).reshape(NTOK, HF)

    in_maps = []
    for cid in range(N_CORES):
        sl = slice(cid * TOK, (cid + 1) * TOK)
        in_maps.append(
            {
                "uT": _round_fp32r(u_all[sl].T),           # (HF, TOK)
                "vT": _round_fp32r(v_all[sl].T),
                "wU": wu,
                "wV": wv,
                "betaU": bu,
                "betaV": bv,
                "ones": ones,
            }
        )

    nc = _get_nc()
    kw = dict(_spmd_kwargs or {})
    one_core = kw.pop("_one_core", False)
    if one_core:
        res = run_bass_kernel_spmd(nc, in_maps[:1], core_ids=[0], **kw)
        return None, res
    res = run_bass_kernel_spmd(
        nc, in_maps, core_ids=list(range(N_CORES)), **kw
    )
    def reassemble(a):
        # (NT, MT, 2, 128, NW) -> (TOK, OUT_F)
        a = a.reshape(NT, MT, 2, 128, JB, H)
        return a.transpose(1, 3, 0, 4, 2, 5).reshape(TOK, OUT_F)

    y = np.concatenate([reassemble(r["y"]) for r in res.results], axis=0)
    out = y.reshape(BATCH, SEQ, OUT_F)
    if _spmd_kwargs:
        return out, res
    return out
